# revision 8
# baseline (speedup 1.0000x reference)
"""MultiHeadCrossModalAttention TRN2 kernel (8 NeuronCores, self-contained).

Problem (hardcoded): B=4, S=2048, D=512, H=8, HD=64, fp32.
  Q = heads(mod1 @ Wq + bq); K/V/scale/shift = heads(mod2 @ W* + b*)
  K = K*scale+shift; V = V*scale+shift
  out = softmax(Q K^T / 8) V  -> concat heads -> @ Wo + bo

Sharding: core c handles batch b=c//2 and head-group g=c%2 (4 heads,
256 feature cols). The output projection is row-split over head groups,
so each core produces a partial [S, D] product; the host sums the two
partials per batch (exact fp32 add) to unshard.

On-chip layout: activations kept TRANSPOSED ([feature, seq]) so every
matmul's contraction dim sits on partitions. Scores are computed
transposed per head ([k, q]); softmax over the partition (k) axis gets
its denominator from a ones-column appended to V (row 64 of the attn
psum accumulates sum_k P). Heads are processed in pairs occupying PE
row-groups 0-63 / 64-127 so their K=64 score matmuls pack into the
128-row array concurrently, and one [h0|h1] 1024-wide exp serves both.
Matmuls run in float32r (full PE rate, ~1.5e-4 rounding); softmax,
FiLM, biases in fp32 on DVE/ACT; exp is the only ACT table function.
DMA traffic is split across the two HW-DGE queues (sync: x2 + output,
scalar: x1 + weights).
"""
import numpy as np
import concourse.bass as bass
import concourse.mybir as mybir
import concourse.tile as tile
from concourse import bacc
from concourse.bass_utils import run_bass_kernel_spmd
from concourse.masks import make_identity
from contextlib import ExitStack

F32 = mybir.dt.float32
F32R = mybir.dt.float32r
AF = mybir.ActivationFunctionType
OP = mybir.AluOpType

B, S, D, H = 4, 2048, 512, 8
HD = 64          # head dim
NG = 256         # feature cols per head-group (4 heads)
NH = 4           # heads per group
ST = S // 128    # 16 s-tiles
DB = D // 128    # 4 d-blocks
KT = S // 128    # 16 k-tiles
N_CORES = 8


def build():
    nc = bacc.Bacc(None)
    x1 = nc.dram_tensor("x1", [S, D], F32, kind="ExternalInput")
    x2 = nc.dram_tensor("x2", [S, D], F32, kind="ExternalInput")
    w_in = {}
    b_in = {}
    for p in ("q", "k", "v", "s", "sh"):
        w_in[p] = nc.dram_tensor(f"w{p}", [D, NG], F32R, kind="ExternalInput")
        b_in[p] = nc.dram_tensor(f"b{p}", [NG], F32, kind="ExternalInput")
    wo = nc.dram_tensor("wo", [NG, D], F32R, kind="ExternalInput")
    bo = nc.dram_tensor("bo", [D], F32, kind="ExternalInput")
    out = nc.dram_tensor("out", [S, D], F32, kind="ExternalOutput")

    with tile.TileContext(nc) as tc, ExitStack() as top:
        cst = top.enter_context(tc.tile_pool(name="cst", bufs=1))
        ident = cst.tile([128, 128], F32, tag="ident", name="ident")
        make_identity(nc, ident)
        identr = cst.tile([128, 128], F32R, tag="identr", name="identr")
        nc.vector.tensor_copy(identr, ident)
        ones16 = cst.tile([128, 16], F32, tag="ones16", name="ones16")
        nc.vector.memset(ones16, 1.0)
        # bo broadcast to 128 partitions (added to every out s-tile)
        bo_row = cst.tile([1, D], F32, tag="bo_row", name="bo_row")
        nc.sync.dma_start(bo_row, bo[:].rearrange("(o n) -> o n", o=1))
        bo_bc = cst.tile([128, D], F32, tag="bo_bc", name="bo_bc")
        nc.gpsimd.partition_broadcast(bo_bc, bo_row)
        # per-partition bias columns [128,1] x 2 row-tiles per projection
        bias = {}
        for p in ("q", "k", "v", "s", "sh"):
            for r in range(2):
                t = cst.tile([128, 1], F32, tag=f"b{p}{r}", name=f"b{p}{r}")
                nc.sync.dma_start(
                    t, b_in[p][r * 128:(r + 1) * 128].rearrange("(p o) -> p o", o=1))
                bias[(p, r)] = t
        wo_t = []
        for r in range(2):
            t = cst.tile([128, D], F32R, tag=f"wo{r}", name=f"wo{r}")
            nc.scalar.dma_start(t, wo[r * 128:(r + 1) * 128, :])
            wo_t.append(t)

        # ---- persistent activation tensors (2 row-tiles of 128 each) ----
        actp = top.enter_context(tc.tile_pool(name="actp", bufs=1))
        Qb = [actp.tile([128, S], F32R, tag=f"Qb{r}", name=f"Qb{r}") for r in range(2)]
        Ktf = [actp.tile([128, S], F32R, tag=f"Ktf{r}", name=f"Ktf{r}") for r in range(2)]

        with tc.tile_pool(name="fp2", bufs=1) as fp2:
          Vtf = [fp2.tile([128, S], F32R, tag=f"Vtf{r}", name=f"Vtf{r}") for r in range(2)]
          with tc.tile_pool(name="fp1", bufs=1) as fp1:
            Sb = [fp1.tile([128, S], F32R, tag=f"Sb{r}", name=f"Sb{r}") for r in range(2)]
            Shb = [fp1.tile([128, S], F32R, tag=f"Shb{r}", name=f"Shb{r}") for r in range(2)]
            with tc.tile_pool(name="xp", bufs=1) as xp, \
                 tc.tile_pool(name="wp", bufs=1) as wp:
                x1t = [xp.tile([128, S], F32R, tag=f"x1t{d}", name=f"x1t{d}") for d in range(DB)]
                x2t = [xp.tile([128, S], F32R, tag=f"x2t{d}", name=f"x2t{d}") for d in range(DB)]

                # ---- P1: transpose inputs into [d, s] layout.
                # Groups of 4 s-tiles: 16 PE transposes fill 4 one-bank psum
                # tiles; one [128,512] DVE copy drains each.
                with tc.tile_pool(name="natp", bufs=6) as natp, \
                     tc.tile_pool(name="trp", bufs=4, space="PSUM") as trp:
                    for src_dram, dst, dma_eng in ((x2, x2t, nc.sync),
                                                   (x1, x1t, nc.scalar)):
                        for sg in range(ST // 4):
                            nats = []
                            for j in range(4):
                                nat = natp.tile([128, D], F32, tag="nat", name="nat")
                                st = sg * 4 + j
                                dma_eng.dma_start(
                                    nat, src_dram[st * 128:(st + 1) * 128, :])
                                nats.append(nat)
                            for d in range(DB):
                                pst = trp.tile([128, 512], F32, tag="trps",
                                               name="pst")
                                for j in range(4):
                                    nc.tensor.transpose(
                                        pst[:, j * 128:(j + 1) * 128],
                                        nats[j][:, d * 128:(d + 1) * 128], ident)
                                nc.vector.tensor_copy(
                                    dst[d][:, sg * 512:(sg + 1) * 512], pst)

                    # ---- P2: five projections, transposed outputs ----
                    with tc.tile_pool(name="pjp", bufs=3, space="PSUM") as pjp:
                        for p in ("s", "sh", "q", "k", "v"):
                            src = x1t if p == "q" else x2t
                            wts = []
                            for d in range(DB):
                                wt = wp.tile([128, NG], F32R, tag=f"w{p}{d}", name=f"w{p}{d}")
                                nc.scalar.dma_start(wt, w_in[p][d * 128:(d + 1) * 128, :])
                                wts.append(wt)
                            for r in range(2):
                                for sc in range(4):
                                    ps = pjp.tile([128, 512], F32, tag="pj", name="pj")
                                    for d in range(DB):
                                        nc.tensor.matmul(
                                            ps,
                                            wts[d][:, r * 128:(r + 1) * 128],
                                            src[d][:, sc * 512:(sc + 1) * 512],
                                            start=(d == 0), stop=(d == DB - 1))
                                    col = slice(sc * 512, (sc + 1) * 512)
                                    if p == "q":
                                        nc.vector.tensor_scalar_add(
                                            Qb[r][:, col], ps, bias[("q", r)])
                                    elif p == "s":
                                        nc.vector.tensor_scalar_add(
                                            Sb[r][:, col], ps, bias[("s", r)])
                                    elif p == "sh":
                                        nc.vector.tensor_scalar_add(
                                            Shb[r][:, col], ps, bias[("sh", r)])
                                    else:
                                        t1 = fp1.tile([128, 512], F32, tag="t1",
                                                      name="t1", bufs=2)
                                        nc.vector.scalar_tensor_tensor(
                                            t1, ps, bias[(p, r)],
                                            Sb[r][:, col].bitcast(F32),
                                            op0=OP.add, op1=OP.mult)
                                        dst = Ktf if p == "k" else Vtf
                                        nc.vector.tensor_tensor(
                                            dst[r][:, col], t1,
                                            Shb[r][:, col].bitcast(F32),
                                            op=OP.add)

          # ---- P4: V^T -> V natural, one [128, 16*65] tile per head with
          # a ones column in slot 64 of each 65-wide k-tile block.
          with tc.tile_pool(name="vgp", bufs=1) as vgp:
            vaug = []
            with tc.tile_pool(name="tr2p", bufs=4, space="PSUM") as tr2p:
                for h in range(NH):
                    vt = vgp.tile([128, KT * 65], F32R, tag=f"vg{h}",
                                  name=f"vg{h}")
                    vaug.append(vt)
                    # ones columns: dst AP [128, 16, 1] strided by 65
                    nc.vector.tensor_copy(
                        vt.rearrange("p (k c) -> p k c", c=65)[:, :, 64:65],
                        ones16.rearrange("p (k o) -> p k o", o=1))
                for j in range(2):          # head pair (2j, 2j+1)
                    for kg in range(KT // 4):   # 4 k-tiles per psum batch
                        pv = [None, None]
                        for hi in range(2):
                            o = 64 * hi
                            pv[hi] = tr2p.tile([128, 256], F32R,
                                               tag=f"tr2{hi}", name="pv")
                            for j4 in range(4):
                                kt = kg * 4 + j4
                                nc.tensor.transpose(
                                    pv[hi][:, j4 * 64:(j4 + 1) * 64],
                                    Vtf[j][o:o + 64, kt * 128:(kt + 1) * 128],
                                    identr[o:o + 64, o:o + 64])
                        for hi in range(2):
                            h = 2 * j + hi
                            nc.vector.tensor_copy(
                                vaug[h].rearrange(
                                    "p (k c) -> p k c", c=65
                                )[:, kg * 4:(kg + 1) * 4, 0:64],
                                pv[hi].bitcast(F32).rearrange(
                                    "p (k c) -> p k c", c=64))
            # ---- P5: attention. Head pairs share the PE array (rows
            # 0-63 / 64-127) and one [h0|h1] exp per k-tile.
            with tc.tile_pool(name="atp", bufs=1) as atp:
                At = [atp.tile([128, S], F32R, tag=f"At{r}", name=f"At{r}")
                      for r in range(2)]
                with tc.tile_pool(name="ptp", bufs=3) as ptp, \
                     tc.tile_pool(name="dnp", bufs=3) as dnp, \
                     tc.tile_pool(name="sps", bufs=3, space="PSUM") as sps, \
                     tc.tile_pool(name="ops", bufs=1, space="PSUM") as ops:
                    for j in range(2):
                        for qc in range(4):     # 512-wide q chunks
                            q_sl = slice(qc * 512, (qc + 1) * 512)
                            o_ps = [ops.tile([65, 512], F32, tag=f"o{hi}",
                                             name="o_ps") for hi in range(2)]
                            for kt in range(KT):
                                stp = sps.tile([128, 1024], F32, tag="sps",
                                               name="stp")
                                for hi in range(2):
                                    o = 64 * hi
                                    nc.tensor.matmul(
                                        stp[:, hi * 512:(hi + 1) * 512],
                                        Ktf[j][o:o + 64, kt * 128:(kt + 1) * 128],
                                        Qb[j][o:o + 64, q_sl],
                                        start=True, stop=True)
                                pt = ptp.tile([128, 1024], F32R, tag="pt",
                                              name="pt")
                                nc.scalar.activation(pt, stp, AF.Exp, scale=0.125)
                                for hi in range(2):
                                    nc.tensor.matmul(
                                        o_ps[hi],
                                        vaug[2 * j + hi][:, kt * 65:kt * 65 + 65],
                                        pt[:, hi * 512:(hi + 1) * 512],
                                        start=(kt == 0), stop=(kt == KT - 1))
                            for hi in range(2):
                                dn = dnp.tile([1, 512], F32, tag="dn", name="dn")
                                nc.vector.reciprocal(dn, o_ps[hi][64:65, :])
                                bc = dnp.tile([64, 512], F32, tag="bc", name="bc")
                                nc.gpsimd.partition_broadcast(bc, dn)
                                nc.vector.tensor_tensor(
                                    At[j][64 * hi:64 * hi + 64, q_sl],
                                    o_ps[hi][0:64, :], bc, op=OP.mult)

                # ---- P6: output projection (row-split partial) + bias ----
                with tc.tile_pool(name="osb", bufs=3) as osb, \
                     tc.tile_pool(name="pso", bufs=2, space="PSUM") as pso:
                    for st in range(ST):
                        op_ps = pso.tile([128, D], F32, tag="pso", name="op_ps")
                        for r in range(2):
                            nc.tensor.matmul(
                                op_ps, At[r][:, st * 128:(st + 1) * 128], wo_t[r],
                                start=(r == 0), stop=(r == 1))
                        ot = osb.tile([128, D], F32, tag="ot", name="ot")
                        nc.vector.tensor_tensor(ot, op_ps, bo_bc, op=OP.add)
                        nc.sync.dma_start(out[st * 128:(st + 1) * 128, :], ot)

    nc.compile()
    return nc


_NC = None


def kernel(mod1_feat, mod2_feat, Wq, bq, Wk, bk, Wv, bv, Wo, bo, Ws, bs,
           Wsh, bsh):
    global _NC
    if _NC is None:
        _NC = build()
    zeros_bo = np.zeros_like(bo)
    in_maps = []
    for c in range(N_CORES):
        b, g = c // 2, c % 2
        cols = slice(g * NG, (g + 1) * NG)
        in_maps.append({
            "x1": np.ascontiguousarray(mod1_feat[b]),
            "x2": np.ascontiguousarray(mod2_feat[b]),
            "wq": np.ascontiguousarray(Wq[:, cols]),
            "bq": np.ascontiguousarray(bq[cols]),
            "wk": np.ascontiguousarray(Wk[:, cols]),
            "bk": np.ascontiguousarray(bk[cols]),
            "wv": np.ascontiguousarray(Wv[:, cols]),
            "bv": np.ascontiguousarray(bv[cols]),
            "ws": np.ascontiguousarray(Ws[:, cols]),
            "bs": np.ascontiguousarray(bs[cols]),
            "wsh": np.ascontiguousarray(Wsh[:, cols]),
            "bsh": np.ascontiguousarray(bsh[cols]),
            "wo": np.ascontiguousarray(Wo[cols, :]),
            "bo": bo if g == 0 else zeros_bo,
        })
    res = run_bass_kernel_spmd(_NC, in_maps, list(range(N_CORES)))
    outs = [res.results[c]["out"] for c in range(N_CORES)]
    full = np.stack([outs[2 * b] + outs[2 * b + 1] for b in range(B)])
    return full.astype(np.float32)
